# revision 15
# baseline (speedup 1.0000x reference)
"""Trainium2 Bass kernel for the nms_detection head (nn_DFD_43327630082789).

Computation (per batch element b, data-parallel over 8 cores):
  x[N,256]   = concat of 5 feature maps, flattened+transposed (N = 21824 tokens)
  h[N,512]   = relu(x @ [cls_Wh; box_Wh].T + [cls_bh; box_bh])
  head[N,84] = h @ W_head.T + b_head  (block-diag: cls rows use h[:256], box rows h[256:])
  out[:, :80]  = cls logits
  out[:, 80:82] = prior_cxcy + 0.1 * tanh(box[:, :2])
  out[:, 82:84] = prior_wh * 2**tanh(box[:, 2:])

Layout strategy: channel-major everywhere. fm.reshape(256, H*W) is already x^T,
so the kernel runs with channels on SBUF partitions and tokens on the free dim —
zero on-device transposes. The box rows are staged raw to a DRAM scratch and
decoded in one packed [128, 688] pass (4 rows x 32 column-chunks on partitions)
so the tiny 4-row elementwise work doesn't waste 97% of the engine lanes.

The subchunk loop is software-pipelined: head matmuls for subchunk s-1 are
emitted between layer-1 matmul groups of subchunk s so the PE never waits for
the (slower) scalar/vector relu copies that produce its head operands.
"""

import os

import ml_dtypes
import numpy as np

import concourse.bass as bass  # noqa: F401  (bass must import before tile)
import concourse.mybir as mybir
import concourse.tile as tile
from concourse import bacc
from concourse.bass_utils import run_bass_kernel_spmd

# Problem constants (hardcoded per contract - no spec.json access here)
B = 8
FEAT = 256
HID2 = 512  # cls hidden (256) + box hidden (256)
NCLS = 80
NOUT = 84  # 80 cls + 4 box
SIZES = [(128, 128), (64, 64), (32, 32), (16, 16), (8, 8)]
N = sum(h * w for h, w in SIZES)  # 21824
NP = 22016  # padded to 43*512
PACK_C = 32
PACK_F = NP // PACK_C  # 688
DELTA_XY = 0.1
LN2 = float(np.log(2.0))

SUB = 1024  # subchunk width (psum tile free dim)
OUTER = 4096  # x DMA granularity

F32 = mybir.dt.float32
BF16 = mybir.dt.bfloat16
F8 = mybir.dt.float8e4
AF = mybir.ActivationFunctionType
ALU = mybir.AluOpType

# "bf16": all-bf16 matmuls (err ~5.1e-4)
# "fp8x": fp8 x + DoubleRow layer-1, bf16 head (err ~5.7e-3)
# "fp8":  fp8 x and h, DoubleRow both layers (err ~8.0e-3)
PRECISION = os.environ.get("KERNEL_PRECISION", "fp8x")

_CACHE: dict = {}
LAST_RESULTS = None  # test.py reads exec_time_ns off this


def _build_nc(repeat: int = 1, dynamic_repeat: bool = False, precision: str = None):
    precision = precision or PRECISION
    xdt = F8 if precision in ("fp8x", "fp8") else BF16
    hdt = F8 if precision == "fp8" else BF16
    nc = bacc.Bacc("TRN2", target_bir_lowering=False, debug=False)

    xT = nc.dram_tensor("xT", [FEAT, NP], xdt, kind="ExternalInput")
    w1t = nc.dram_tensor("w1t", [FEAT, HID2], xdt, kind="ExternalInput")
    wht = nc.dram_tensor("wht", [HID2, NOUT], hdt, kind="ExternalInput")
    b1 = nc.dram_tensor("b1", [128, 4], F32, kind="ExternalInput")
    bh = nc.dram_tensor("bh", [NOUT, 1], F32, kind="ExternalInput")
    pri = nc.dram_tensor("priors", [128, PACK_F], F32, kind="ExternalInput")
    outT = nc.dram_tensor("outT", [NOUT, NP], F32, kind="ExternalOutput")

    with tile.TileContext(nc) as tc:
        with (
            tc.tile_pool(name="const", bufs=1) as cp,
            tc.tile_pool(name="xp", bufs=3) as xp,
            tc.tile_pool(name="hp", bufs=3) as hpool,
            tc.tile_pool(name="op", bufs=3) as opool,
            tc.tile_pool(name="bx", bufs=1) as bpool,
            tc.tile_pool(name="dramp", bufs=1, space="DRAM") as dpool,
            tc.tile_pool(name="psh", bufs=2, space="PSUM") as psh,
            tc.tile_pool(name="pso", bufs=2, space="PSUM") as pso,
        ):
            w1sb = cp.tile([128, 2, HID2], xdt)
            nc.sync.dma_start(w1sb[:], w1t.ap().rearrange("(kc p) h -> p kc h", p=128))
            whsb = cp.tile([128, 4, NOUT], hdt)
            nc.sync.dma_start(whsb[:], wht.ap().rearrange("(kc p) o -> p kc o", p=128))
            b1sb = cp.tile([128, 4], F32)
            nc.sync.dma_start(b1sb[:], b1.ap())
            bhsb = cp.tile([NOUT, 1], F32)
            nc.sync.dma_start(bhsb[:], bh.ap())
            prisb = cp.tile([128, PACK_F], F32)
            nc.sync.dma_start(prisb[:], pri.ap())

            scratch = dpool.tile([4, NP], F32)  # raw box logits staging

            xT_r = xT.ap().rearrange("(kc p) n -> p kc n", p=128)

            def emit_head(st):
                """Head matmuls + bias copy + output DMAs for one subchunk."""
                hsb, col0, W, flip = st
                ops = pso.tile([NOUT, SUB], F32, tag="ops")
                if hdt == F8:
                    for k in (0, 2):
                        for h0 in range(0, W, 512):
                            hw = min(512, W - h0)
                            nc.tensor.matmul(
                                ops[:, h0 : h0 + hw],
                                whsb[:, k : k + 2, :],
                                hsb[:, k : k + 2, h0 : h0 + hw],
                                start=(k == 0),
                                stop=(k == 2),
                                perf_mode=mybir.MatmulPerfMode.DoubleRow,
                            )
                else:
                    for k in range(4):
                        for h0 in range(0, W, 512):
                            hw = min(512, W - h0)
                            nc.tensor.matmul(
                                ops[:, h0 : h0 + hw],
                                whsb[:, k, :],
                                hsb[:, k, h0 : h0 + hw],
                                start=(k == 0),
                                stop=(k == 3),
                            )
                osb = opool.tile([NOUT, SUB], F32, tag="osb")
                if flip % 2 == 0:
                    nc.scalar.activation(
                        osb[:, :W], ops[:, :W], AF.Identity, bias=bhsb[:, 0:1]
                    )
                else:
                    nc.vector.tensor_scalar(
                        osb[:, :W], ops[:, :W], bhsb[:, 0:1], None, ALU.add
                    )
                nc.sync.dma_start(
                    outT.ap()[0:NCLS, col0 : col0 + W], osb[0:NCLS, :W]
                )
                nc.sync.dma_start(
                    scratch[0:4, col0 : col0 + W], osb[NCLS:NOUT, :W]
                )

            def emit_body():
                # flat subchunk schedule: (xin tile, offset in xin, width, col0)
                sched = []
                col0 = 0
                while col0 < NP:
                    ow = min(OUTER, NP - col0)
                    xin = xp.tile([128, 2, OUTER], xdt, tag="xin")
                    nc.sync.dma_start(
                        xin[:, :, :ow], xT_r[:, :, col0 : col0 + ow]
                    )
                    for s0 in range(0, ow, SUB):
                        sched.append((xin, s0, min(SUB, ow - s0), col0 + s0))
                    col0 += ow

                pending = None
                for idx, (xin, s0, W, gcol) in enumerate(sched):
                    hsb = hpool.tile([128, 4, SUB], hdt, tag="hsb")
                    for m in range(4):
                        hps = psh.tile([128, SUB], F32, tag="hps")
                        if xdt == F8:
                            for h0 in range(0, W, 512):
                                hw = min(512, W - h0)
                                nc.tensor.matmul(
                                    hps[:, h0 : h0 + hw],
                                    w1sb[:, 0:2, m * 128 : (m + 1) * 128],
                                    xin[:, 0:2, s0 + h0 : s0 + h0 + hw],
                                    start=True,
                                    stop=True,
                                    perf_mode=mybir.MatmulPerfMode.DoubleRow,
                                )
                        else:
                            for k in range(2):
                                for h0 in range(0, W, 512):
                                    hw = min(512, W - h0)
                                    nc.tensor.matmul(
                                        hps[:, h0 : h0 + hw],
                                        w1sb[:, k, m * 128 : (m + 1) * 128],
                                        xin[:, k, s0 + h0 : s0 + h0 + hw],
                                        start=(k == 0),
                                        stop=(k == 1),
                                    )
                        # fused psum->sbuf copy + bias + relu (+ bf16 cast)
                        if m % 2 == 0:
                            nc.scalar.activation(
                                hsb[:, m, :W], hps[:, :W], AF.Relu,
                                bias=b1sb[:, m : m + 1],
                            )
                        else:
                            nc.vector.tensor_scalar(
                                hsb[:, m, :W], hps[:, :W],
                                b1sb[:, m : m + 1], 0.0, ALU.add, ALU.max,
                            )
                        if m == 1 and pending is not None:
                            emit_head(pending)
                            pending = None
                    pending = (hsb, gcol, W, idx)
                if pending is not None:
                    emit_head(pending)

                # --- packed box decode: [4, NP] viewed as [128, 688] ---
                braw = bpool.tile([128, PACK_F], F32)
                nc.sync.dma_start(
                    braw[:], scratch[:].rearrange("r (c f) -> (r c) f", c=PACK_C)
                )
                bd = bpool.tile([128, PACK_F], F32)
                nc.scalar.activation(bd[:], braw[:], AF.Tanh)
                bex = bpool.tile([128, PACK_F], F32)
                nc.scalar.activation(bex[:], bd[:], AF.Exp, scale=LN2)
                bout = bpool.tile([128, PACK_F], F32)
                # partitions 0:64 = cx,cy rows: prior + 0.1*tanh(d)
                nc.vector.scalar_tensor_tensor(
                    bout[0:64], bd[0:64], DELTA_XY, prisb[0:64], ALU.mult, ALU.add
                )
                # partitions 64:128 = w,h rows: prior * 2**tanh(d)
                nc.vector.tensor_tensor(
                    bout[64:128], bex[64:128], prisb[64:128], ALU.mult
                )
                nc.sync.dma_start(
                    outT.ap()[NCLS:NOUT, :].rearrange("r (c f) -> (r c) f", c=PACK_C),
                    bout[:],
                )

            if dynamic_repeat and repeat > 1:
                ET = mybir.EngineType
                hints = (
                    (ET.PE, ET.Activation, ET.DVE, ET.SP)
                    if os.environ.get("KERNEL_LOOP_HINTS", "0") == "1"
                    else ()
                )
                with tc.For_i(0, repeat, 1, hint_engines=hints):
                    emit_body()
            else:
                for _rep in range(repeat):
                    emit_body()

    nc.compile()
    return nc


def _priors_packed() -> np.ndarray:
    rows = np.zeros((4, NP), np.float32)
    rows[2:4, :] = 1.0  # pad region: any finite value
    off = 0
    for h, w in SIZES:
        cx = (np.arange(w, dtype=np.float32) + np.float32(0.5)) / np.float32(w)
        cy = (np.arange(h, dtype=np.float32) + np.float32(0.5)) / np.float32(h)
        n = h * w
        rows[0, off : off + n] = np.tile(cx, h)
        rows[1, off : off + n] = np.repeat(cy, w)
        rows[2, off : off + n] = np.float32(1.0) / np.float32(w)
        rows[3, off : off + n] = np.float32(1.0) / np.float32(h)
        off += n
    return rows.reshape(4, PACK_C, PACK_F).reshape(128, PACK_F).copy()


def prep_inputs(fm0, fm1, fm2, fm3, fm4,
                cls_Wh, cls_bh, cls_Wo, cls_bo,
                box_Wh, box_bh, box_Wo, box_bo,
                precision: str = None):
    """Host-side shard + pack: returns in_maps (one dict per core)."""
    precision = precision or PRECISION
    fms = [np.asarray(f, np.float32) for f in (fm0, fm1, fm2, fm3, fm4)]
    bf = ml_dtypes.bfloat16
    f8 = ml_dtypes.float8_e4m3
    xnp = f8 if precision in ("fp8x", "fp8") else bf
    hnp = f8 if precision == "fp8" else bf

    # layer-1 lhsT: [FEAT, 512] = concat(cls_Wh, box_Wh).T
    w1t = np.ascontiguousarray(
        np.concatenate([np.asarray(cls_Wh), np.asarray(box_Wh)], axis=0).T
    ).astype(xnp)
    # head lhsT: [512, 84], block-diagonal
    w_head = np.zeros((NOUT, HID2), np.float32)
    w_head[0:NCLS, 0:256] = np.asarray(cls_Wo)
    w_head[NCLS:NOUT, 256:512] = np.asarray(box_Wo)
    wht = np.ascontiguousarray(w_head.T).astype(hnp)

    b1 = np.ascontiguousarray(
        np.concatenate([np.asarray(cls_bh), np.asarray(box_bh)])
        .astype(np.float32).reshape(4, 128).T
    )
    bhead = np.ascontiguousarray(
        np.concatenate([np.asarray(cls_bo), np.asarray(box_bo)])
        .astype(np.float32).reshape(NOUT, 1)
    )
    priors = _priors_packed()

    in_maps = []
    for b in range(B):
        xt = np.zeros((FEAT, NP), xnp)
        off = 0
        for f in fms:
            hw = f.shape[2] * f.shape[3]
            xt[:, off : off + hw] = f[b].reshape(FEAT, hw).astype(xnp)
            off += hw
        in_maps.append({
            "xT": xt, "w1t": w1t, "wht": wht,
            "b1": b1, "bh": bhead, "priors": priors,
        })
    return in_maps


def kernel(fm0, fm1, fm2, fm3, fm4,
           cls_Wh, cls_bh, cls_Wo, cls_bo,
           box_Wh, box_bh, box_Wo, box_bo):
    global LAST_RESULTS
    key = ("nc", PRECISION)
    if key not in _CACHE:
        _CACHE[key] = _build_nc()
    nc = _CACHE[key]

    in_maps = prep_inputs(fm0, fm1, fm2, fm3, fm4,
                          cls_Wh, cls_bh, cls_Wo, cls_bo,
                          box_Wh, box_bh, box_Wo, box_bo)

    res = run_bass_kernel_spmd(
        nc, in_maps, core_ids=list(range(B)),
        trace=bool(int(os.environ.get("KERNEL_TRACE", "0"))),
    )
    LAST_RESULTS = res

    out = np.empty((B, N, NOUT), np.float32)
    for b in range(B):
        out[b] = res.results[b]["outT"][:, :N].T
    return out
